# revision 21
# baseline (speedup 1.0000x reference)
"""Trainium2 Bass kernel for nn_BackProjLayer.

Math: S = M M^H/Nch + 1e-3 I is strictly PD, so the reference's
eigh -> clamp -> scale -> |D^H V|^2 pipeline is exactly
latent[b,p] = Re(D[:,p]^H S_b D[:,p]) — a quadratic form. Exploiting
Hermitian symmetry, each S_b is 1024 independent reals (32 diag +
496 upper-re + 496 upper-im), so:

  latent = SP @ K1          SP: (B,1024) packed S, K1: (1024,484)
  h1     = relu(conv1(latent) + b1)      (conv1 folded into K1 host-side)
  x      = relu(conv2(h1) + b2)          (conv2 = banded matmul on device)
  OUT    = x @ GP^T         GP: (1024,484) packed-Hermitian A diag(x) A^H

Device layout: feature-major (features/pixels on partitions, batch on
free dim). Pixels are chunked 4 x 121 with +/-1 halo so conv2 is a
single K=128 matmul per chunk. Everything runs on TensorE at N=512.

Sharding: pure data-parallel over batch, 1024 rows per core, 8 cores.
"""

import numpy as np

B, NCH, NPX = 8192, 32, 484
NCORES = 8
BLOC = B // NCORES          # 1024 batch rows per core
NB = 512                    # batch-phase width (matmul N)
NHALF = BLOC // NB          # batch phases
NCHUNK, CH = 4, 121         # pixel chunks: 4 x 121 = 484
HALO = CH + 2               # in-pixels per chunk incl. halo
KF = 1024                   # packed-S feature count (= NCH + 2*496)
KO = KF // 128              # contraction chunks for MM1

# matmul operand dtype: "float32" (4 cyc/row) or "float32r" (1 cyc/row @ N>=256)
MM_DTYPE = "float32r"


def _build_host_weights(A, D, w1, b1, w2, b2):
    """All-fp64 host preprocessing of the constant operands."""
    iu = np.triu_indices(NCH, 1)
    # T[p,c,c'] = conj(D[c,p]) D[c',p];  latent = sum T .* S (Hermitian pair)
    T = np.conj(D).T[:, :, None] * D.T[:, None, :]          # (NPX, NCH, NCH)
    K1 = np.concatenate([
        np.real(T[:, np.arange(NCH), np.arange(NCH)]),
        2 * np.real(T[:, iu[0], iu[1]]),
        -2 * np.imag(T[:, iu[0], iu[1]]),
    ], axis=1).T                                            # (1024, NPX)
    # fold conv1 (kernel 3, zero-pad) along the pixel axis
    K1pad = np.pad(K1, ((0, 0), (1, 1)))
    K1c = w1[0] * K1pad[:, :-2] + w1[1] * K1pad[:, 1:-1] + w1[2] * K1pad[:, 2:]
    # halo'd MM1 weights: W1h[k, c, j] = K1c[k, 121c-1+j], zero outside
    W1h = np.zeros((KF, NCHUNK, 128))
    for c in range(NCHUNK):
        for j in range(HALO):
            q = CH * c - 1 + j
            if 0 <= q < NPX:
                W1h[:, c, j] = K1c[:, q]
    # conv2 band matrices, M padded to 128 (zero cols -> defined zero output)
    C2 = np.zeros((NCHUNK, 128, 128))
    for c in range(NCHUNK):
        for j in range(HALO):
            q = CH * c - 1 + j
            if not (0 <= q < NPX):
                continue
            for o in range(CH):
                t = j - o
                if 0 <= t <= 2:
                    C2[c, j, o] = w2[t]
    # back-projection, packed Hermitian: G[i,l,p] = A[i,p] conj(A[l,p])
    G = A[:, None, :] * np.conj(A)[None, :, :]
    GPfull = np.concatenate([
        np.real(G[np.arange(NCH), np.arange(NCH), :]),
        np.real(G[iu[0], iu[1], :]),
        np.imag(G[iu[0], iu[1], :]),
    ], axis=0)                                              # (1024, NPX)
    GPh = np.zeros((NCHUNK, 128, KF))
    for c in range(NCHUNK):
        GPh[c, :CH, :] = GPfull[:, CH * c:CH * c + CH].T
    return W1h, C2, GPh


def _pack_S(S):
    """S: (B,1,32,32) complex -> (B, 1024) float32 packed Hermitian."""
    S = np.ascontiguousarray(S[:, 0])
    iu = np.triu_indices(NCH, 1)
    return np.concatenate([
        np.real(S[:, np.arange(NCH), np.arange(NCH)]),
        np.real(S[:, iu[0], iu[1]]),
        np.imag(S[:, iu[0], iu[1]]),
    ], axis=1).astype(np.float32)                            # (B, 1024)


def _tf32_round(a):
    """Round fp32 array to tf32 (float32r): round-to-nearest-even at 13 low
    mantissa bits. No-op unless MM_DTYPE is float32r."""
    if MM_DTYPE != "float32r":
        return a.astype(np.float32)
    u = np.ascontiguousarray(a, dtype=np.float32).view(np.uint32).copy()
    u += np.uint32(0x0FFF) + ((u >> np.uint32(13)) & np.uint32(1))
    u &= np.uint32(0xFFFFE000)
    return u.view(np.float32)


def _build_program(b1f, b2f, reps=1, bench_mode='full'):
    import concourse.bacc as bacc
    import concourse.mybir as mybir
    import concourse.tile as tile

    nc = bacc.Bacc(None, target_bir_lowering=False)
    f32 = mybir.dt.float32
    mmdt = getattr(mybir.dt, MM_DTYPE)

    spt_d = nc.dram_tensor("spt", [128, KO, BLOC], mmdt, kind="ExternalInput")
    w1h_d = nc.dram_tensor("w1h", [128, KO, NCHUNK, 128], mmdt, kind="ExternalInput")
    c2_d = nc.dram_tensor("c2", [128, NCHUNK, 128], mmdt, kind="ExternalInput")
    gph_d = nc.dram_tensor("gph", [128, NCHUNK, 8, 128], mmdt, kind="ExternalInput")
    xt_d = nc.dram_tensor("xt", [NCHUNK, 128, BLOC], f32, kind="ExternalOutput")
    outt_d = nc.dram_tensor("outt", [8, 128, BLOC], f32, kind="ExternalOutput")

    relu = mybir.ActivationFunctionType.Relu

    with tile.TileContext(nc) as tc:
        with (
            tc.tile_pool(name="consts", bufs=2) as consts,
            tc.tile_pool(name="spt", bufs=2) as spt_pool,
            tc.tile_pool(name="h1p", bufs=6) as h1p,
            tc.tile_pool(name="xb", bufs=10) as xb,
            tc.tile_pool(name="ob", bufs=3) as ob,
            tc.tile_pool(name="ps1", bufs=1, space="PSUM") as ps1,
            tc.tile_pool(name="pso", bufs=4, space="PSUM") as pso,
        ):
          for rep in range(reps):
              # ALL input DMAs issued up front, in consumption order, so
              # the in-order DMA queues are never blocked behind
              # compute-dependent output DMAs. First-needed chunks first:
              # (w1[ko] on ACT, spt0[ko] on SP) pairs feed the very first
              # matmuls. The w1 transfers finish long before the first
              # relu, so they cannot head-of-line-block ACT's compute.
              w1sb = []
              sptsb_all = [[] for _ in range(NHALF)]
              for ko in range(KO):
                  w = consts.tile([128, NCHUNK, 128], mmdt, tag=f"w1_{ko}",
                                  name=f"w1_{ko}_{rep}")
                  nc.scalar.dma_start(w[:], w1h_d[:, ko])
                  w1sb.append(w)
                  t = spt_pool.tile([128, NB], mmdt, tag=f"spt_{ko}",
                                    name=f"spt_{ko}_0_{rep}")
                  nc.sync.dma_start(t[:], spt_d[:, ko, 0:NB])
                  sptsb_all[0].append(t)
              c2sb = consts.tile([128, NCHUNK, 128], mmdt, tag="c2", name=f"c2_{rep}")
              nc.sync.dma_start(c2sb[:], c2_d[:])
              # gph before the second half's S data: PE issues MM2(half0)
              # before MM1(half1), and the in-order queues must match.
              gpsb = []
              for c in range(NCHUNK):
                  t = consts.tile([128, 8, 128], mmdt, tag=f"gph_{c}",
                                  name=f"gph_{c}_{rep}")
                  nc.sync.dma_start(t[:], gph_d[:, c])
                  gpsb.append(t)
              for n in range(1, NHALF):
                  for ko in range(KO):
                      t = spt_pool.tile([128, NB], mmdt, tag=f"spt_{ko}",
                                        name=f"spt_{ko}_{n}_{rep}")
                      nc.sync.dma_start(
                          t[:], spt_d[:, ko, n * NB:(n + 1) * NB]
                      )
                      sptsb_all[n].append(t)

              if bench_mode == 'dma':
                  if rep == 0:
                      dummy = consts.tile([128, NB], f32, tag="dummy")
                      nc.vector.memset(dummy[:], 0.0)
                  for n in range(NHALF):
                      for c in range(NCHUNK):
                          nc.scalar.dma_start(
                              xt_d[c, :, n * NB:(n + 1) * NB], dummy[:])
                      for mc in range(8):
                          nc.sync.dma_start(
                              outt_d[mc, :, n * NB:(n + 1) * NB], dummy[:])
                  continue
              for n in range(NHALF):
                  sptsb = sptsb_all[n]
                  # MM1 ko-major: all 4 chunk psums accumulate in parallel so
                  # the PE only ever waits for one 256KB spt chunk at a time.
                  z1s = [ps1.tile([128, NB], f32, tag=f"z1_{c}", name=f"z1_{c}_{n}_{rep}")
                         for c in range(NCHUNK)]
                  for ko in range(KO):
                      for c in range(NCHUNK):
                          nc.tensor.matmul(
                              z1s[c][:],
                              lhsT=w1sb[ko][:, c, :],
                              rhs=sptsb[ko][:],
                              start=(ko == 0),
                              stop=(ko == KO - 1),
                          )
                  h1s = []
                  for c in range(NCHUNK):
                      h1 = h1p.tile([128, NB], mmdt, tag="h1")
                      if c % 2 == 0:
                          nc.scalar.activation(h1[:], z1s[c][:], relu,
                                               bias=float(b1f))
                      else:
                          nc.vector.tensor_scalar(
                              h1[:], z1s[c][:], float(b1f), 0.0,
                              mybir.AluOpType.add, mybir.AluOpType.max)
                      h1s.append(h1)

                  xs = []
                  for c in range(NCHUNK):
                      # z2 reuses z1_c's bank: the relu that freed it is
                      # exactly the producer of this conv's rhs.
                      z2 = ps1.tile([128, NB], f32, tag=f"z1_{c}",
                                    name=f"z2_{c}_{n}_{rep}")
                      nc.tensor.matmul(
                          z2[:], lhsT=c2sb[:, c, :], rhs=h1s[c][:],
                          start=True, stop=True,
                      )
                      x = xb.tile([128, NB], mmdt, tag="x")
                      if c % 2 == 1:
                          nc.scalar.activation(x[:], z2[:], relu,
                                               bias=float(b2f))
                      else:
                          nc.vector.tensor_scalar(
                              x[:], z2[:], float(b2f), 0.0,
                              mybir.AluOpType.add, mybir.AluOpType.max)
                      nc.scalar.dma_start(
                          xt_d[c, :, n * NB:(n + 1) * NB], x[:].bitcast(f32)
                      )
                      xs.append(x)

                  for mc in range(8):
                      o = pso.tile([128, NB], f32, tag="o")
                      for c in range(NCHUNK):
                          nc.tensor.matmul(
                              o[:],
                              lhsT=gpsb[c][:, mc, :],
                              rhs=xs[c][:],
                              start=(c == 0),
                              stop=(c == NCHUNK - 1),
                          )
                      osb = ob.tile([128, NB], f32, tag="osb")
                      nc.vector.tensor_copy(osb[:], o[:])
                      nc.sync.dma_start(
                          outt_d[mc, :, n * NB:(n + 1) * NB], osb[:]
                      )
    nc.finalize()
    return nc


def kernel(S, A, D, w1, b1, w2, b2, _trace=False, _tmpdir=None, _reps=1):
    from concourse.bass_utils import run_bass_kernel_spmd

    S = np.asarray(S)
    A = np.asarray(A, dtype=np.complex128)
    D = np.asarray(D, dtype=np.complex128)
    w1 = np.asarray(w1, dtype=np.float64)
    w2 = np.asarray(w2, dtype=np.float64)
    b1f = float(np.asarray(b1))
    b2f = float(np.asarray(b2))

    W1h, C2, GPh = _build_host_weights(A, D, w1, b1f, w2, b2f)
    SP = _pack_S(S)                                          # (B, 1024) f32

    # device-layout host arrays (shared across cores except spt)
    w1h_in = _tf32_round(np.ascontiguousarray(
        W1h.reshape(KO, 128, NCHUNK, 128).transpose(1, 0, 2, 3)
    ))                                                       # (128,KO,4,128)
    c2_in = _tf32_round(np.ascontiguousarray(C2.transpose(1, 0, 2)))
    gph_in = _tf32_round(np.ascontiguousarray(
        GPh.reshape(NCHUNK, 128, 8, 128).transpose(1, 0, 2, 3)
    ))                                                       # (128,4,8,128)

    in_maps = []
    for core in range(NCORES):
        sp_c = SP[core * BLOC:(core + 1) * BLOC]             # (BLOC, 1024)
        # spt[p, ko, b] = sp_c[b, ko*128 + p]
        spt = np.ascontiguousarray(
            sp_c.reshape(BLOC, KO, 128).transpose(2, 1, 0)
        )
        in_maps.append({"spt": _tf32_round(spt), "w1h": w1h_in, "c2": c2_in,
                        "gph": gph_in})

    nc = _build_program(b1f, b2f, reps=_reps)
    res = run_bass_kernel_spmd(
        nc, in_maps, core_ids=list(range(NCORES)),
        trace=_trace, tmpdir=_tmpdir,
    )

    # ---- host-side unshard / unpack ----
    iu = np.triu_indices(NCH, 1)
    x_full = np.empty((B, NPX), dtype=np.float64)
    out_full = np.empty((B, NCH, NCH), dtype=np.complex128)
    for core in range(NCORES):
        r = res.results[core]
        sl = slice(core * BLOC, (core + 1) * BLOC)
        xt = r["xt"]                                         # (4,128,BLOC)
        x_full[sl] = np.concatenate(
            [xt[c, :CH, :] for c in range(NCHUNK)], axis=0
        ).T
        outt = r["outt"].reshape(8 * 128, BLOC)              # (1024, BLOC)
        ob = np.empty((BLOC, NCH, NCH), dtype=np.complex128)
        ob[:, np.arange(NCH), np.arange(NCH)] = outt[:NCH].T
        re = outt[NCH:NCH + 496].T
        im = outt[NCH + 496:].T
        ob[:, iu[0], iu[1]] = re + 1j * im
        ob[:, iu[1], iu[0]] = re - 1j * im
        out_full[sl] = ob
    kernel._last_results = res
    return out_full, x_full



# revision 22
# speedup vs baseline: 1.2503x; 1.2503x over previous
"""Trainium2 Bass kernel for nn_BackProjLayer.

Math: S = M M^H/Nch + 1e-3 I is strictly PD, so the reference's
eigh -> clamp -> scale -> |D^H V|^2 pipeline is exactly
latent[b,p] = Re(D[:,p]^H S_b D[:,p]) — a quadratic form. Exploiting
Hermitian symmetry, each S_b is 1024 independent reals (32 diag +
496 upper-re + 496 upper-im), so:

  latent = SP @ K1          SP: (B,1024) packed S, K1: (1024,484)
  h1     = relu(conv1(latent) + b1)      (conv1 folded into K1 host-side)
  x      = relu(conv2(h1) + b2)          (conv2 = banded matmul on device)
  OUT    = x @ GP^T         GP: (1024,484) packed-Hermitian A diag(x) A^H

Device layout: feature-major (features/pixels on partitions, batch on
free dim). Pixels are chunked 4 x 121 with +/-1 halo so conv2 is a
single K=128 matmul per chunk. Everything runs on TensorE at N=512.

Sharding: pure data-parallel over batch, 1024 rows per core, 8 cores.
"""

import numpy as np

B, NCH, NPX = 8192, 32, 484
NCORES = 8
BLOC = B // NCORES          # 1024 batch rows per core
NB = 512                    # batch-phase width (matmul N)
NHALF = BLOC // NB          # batch phases
NCHUNK, CH = 4, 121         # pixel chunks: 4 x 121 = 484
HALO = CH + 2               # in-pixels per chunk incl. halo
KF = 1024                   # packed-S feature count (= NCH + 2*496)
KO = KF // 128              # contraction chunks for MM1

# matmul operand dtype: "float32" (4 cyc/row) or "float32r" (1 cyc/row @ N>=256)
MM_DTYPE = "float32r"


def _build_host_weights(A, D, w1, b1, w2, b2):
    """All-fp64 host preprocessing of the constant operands."""
    iu = np.triu_indices(NCH, 1)
    # T[p,c,c'] = conj(D[c,p]) D[c',p];  latent = sum T .* S (Hermitian pair)
    T = np.conj(D).T[:, :, None] * D.T[:, None, :]          # (NPX, NCH, NCH)
    K1 = np.concatenate([
        np.real(T[:, np.arange(NCH), np.arange(NCH)]),
        2 * np.real(T[:, iu[0], iu[1]]),
        -2 * np.imag(T[:, iu[0], iu[1]]),
    ], axis=1).T                                            # (1024, NPX)
    # fold conv1 (kernel 3, zero-pad) along the pixel axis
    K1pad = np.pad(K1, ((0, 0), (1, 1)))
    K1c = w1[0] * K1pad[:, :-2] + w1[1] * K1pad[:, 1:-1] + w1[2] * K1pad[:, 2:]
    # halo'd MM1 weights: W1h[k, c, j] = K1c[k, 121c-1+j], zero outside
    W1h = np.zeros((KF, NCHUNK, 128))
    for c in range(NCHUNK):
        for j in range(HALO):
            q = CH * c - 1 + j
            if 0 <= q < NPX:
                W1h[:, c, j] = K1c[:, q]
    # conv2 band matrices, M padded to 128 (zero cols -> defined zero output)
    C2 = np.zeros((NCHUNK, 128, 128))
    for c in range(NCHUNK):
        for j in range(HALO):
            q = CH * c - 1 + j
            if not (0 <= q < NPX):
                continue
            for o in range(CH):
                t = j - o
                if 0 <= t <= 2:
                    C2[c, j, o] = w2[t]
    # back-projection, packed Hermitian: G[i,l,p] = A[i,p] conj(A[l,p])
    G = A[:, None, :] * np.conj(A)[None, :, :]
    GPfull = np.concatenate([
        np.real(G[np.arange(NCH), np.arange(NCH), :]),
        np.real(G[iu[0], iu[1], :]),
        np.imag(G[iu[0], iu[1], :]),
    ], axis=0)                                              # (1024, NPX)
    GPh = np.zeros((NCHUNK, 128, KF))
    for c in range(NCHUNK):
        GPh[c, :CH, :] = GPfull[:, CH * c:CH * c + CH].T
    return W1h, C2, GPh


def _pack_S(S):
    """S: (B,1,32,32) complex -> (B, 1024) float32 packed Hermitian.

    Packs (S + S^H)/2, matching eigh's symmetrize_input — bit-identical
    to the plain upper triangle when S is exactly Hermitian."""
    S = np.ascontiguousarray(S[:, 0])
    iu = np.triu_indices(NCH, 1)
    up = S[:, iu[0], iu[1]]
    lo = S[:, iu[1], iu[0]]
    return np.concatenate([
        np.real(S[:, np.arange(NCH), np.arange(NCH)]),
        (np.real(up) + np.real(lo)) * 0.5,
        (np.imag(up) - np.imag(lo)) * 0.5,
    ], axis=1).astype(np.float32)                            # (B, 1024)


def _tf32_round(a):
    """Round fp32 array to tf32 (float32r): round-to-nearest-even at 13 low
    mantissa bits. No-op unless MM_DTYPE is float32r."""
    if MM_DTYPE != "float32r":
        return a.astype(np.float32)
    u = np.ascontiguousarray(a, dtype=np.float32).view(np.uint32).copy()
    u += np.uint32(0x0FFF) + ((u >> np.uint32(13)) & np.uint32(1))
    u &= np.uint32(0xFFFFE000)
    return u.view(np.float32)


def _build_program(b1f, b2f, reps=1, bench_mode='full'):
    import concourse.bacc as bacc
    import concourse.mybir as mybir
    import concourse.tile as tile

    nc = bacc.Bacc(None, target_bir_lowering=False)
    f32 = mybir.dt.float32
    mmdt = getattr(mybir.dt, MM_DTYPE)

    spt_d = nc.dram_tensor("spt", [128, KO, BLOC], mmdt, kind="ExternalInput")
    w1h_d = nc.dram_tensor("w1h", [128, KO, NCHUNK, 128], mmdt, kind="ExternalInput")
    c2_d = nc.dram_tensor("c2", [128, NCHUNK, 128], mmdt, kind="ExternalInput")
    gph_d = nc.dram_tensor("gph", [128, NCHUNK, 8, 128], mmdt, kind="ExternalInput")
    xt_d = nc.dram_tensor("xt", [NCHUNK, 128, BLOC], f32, kind="ExternalOutput")
    outt_d = nc.dram_tensor("outt", [8, 128, BLOC], f32, kind="ExternalOutput")

    relu = mybir.ActivationFunctionType.Relu

    with tile.TileContext(nc) as tc:
        with (
            tc.tile_pool(name="consts", bufs=2) as consts,
            tc.tile_pool(name="spt", bufs=2) as spt_pool,
            tc.tile_pool(name="h1p", bufs=6) as h1p,
            tc.tile_pool(name="xb", bufs=10) as xb,
            tc.tile_pool(name="ob", bufs=3) as ob,
            tc.tile_pool(name="ps1", bufs=1, space="PSUM") as ps1,
            tc.tile_pool(name="pso", bufs=4, space="PSUM") as pso,
        ):
          for rep in range(reps):
              # ALL input DMAs issued up front, in consumption order, so
              # the in-order DMA queues are never blocked behind
              # compute-dependent output DMAs. First-needed chunks first:
              # (w1[ko] on ACT, spt0[ko] on SP) pairs feed the very first
              # matmuls. The w1 transfers finish long before the first
              # relu, so they cannot head-of-line-block ACT's compute.
              w1sb = []
              sptsb_all = [[] for _ in range(NHALF)]
              for ko in range(KO):
                  w = consts.tile([128, NCHUNK, 128], mmdt, tag=f"w1_{ko}",
                                  name=f"w1_{ko}_{rep}")
                  nc.scalar.dma_start(w[:], w1h_d[:, ko])
                  w1sb.append(w)
                  t = spt_pool.tile([128, NB], mmdt, tag=f"spt_{ko}",
                                    name=f"spt_{ko}_0_{rep}")
                  nc.sync.dma_start(t[:], spt_d[:, ko, 0:NB])
                  sptsb_all[0].append(t)
              c2sb = consts.tile([128, NCHUNK, 128], mmdt, tag="c2", name=f"c2_{rep}")
              nc.sync.dma_start(c2sb[:], c2_d[:])
              # gph before the second half's S data: PE issues MM2(half0)
              # before MM1(half1), and the in-order queues must match.
              gpsb = []
              for c in range(NCHUNK):
                  t = consts.tile([128, 8, 128], mmdt, tag=f"gph_{c}",
                                  name=f"gph_{c}_{rep}")
                  nc.sync.dma_start(t[:], gph_d[:, c])
                  gpsb.append(t)
              for n in range(1, NHALF):
                  for ko in range(KO):
                      t = spt_pool.tile([128, NB], mmdt, tag=f"spt_{ko}",
                                        name=f"spt_{ko}_{n}_{rep}")
                      nc.sync.dma_start(
                          t[:], spt_d[:, ko, n * NB:(n + 1) * NB]
                      )
                      sptsb_all[n].append(t)

              if bench_mode == 'dma':
                  if rep == 0:
                      dummy = consts.tile([128, NB], f32, tag="dummy")
                      nc.vector.memset(dummy[:], 0.0)
                  for n in range(NHALF):
                      for c in range(NCHUNK):
                          nc.scalar.dma_start(
                              xt_d[c, :, n * NB:(n + 1) * NB], dummy[:])
                      for mc in range(8):
                          nc.sync.dma_start(
                              outt_d[mc, :, n * NB:(n + 1) * NB], dummy[:])
                  continue
              for n in range(NHALF):
                  sptsb = sptsb_all[n]
                  # MM1 ko-major: all 4 chunk psums accumulate in parallel so
                  # the PE only ever waits for one 256KB spt chunk at a time.
                  z1s = [ps1.tile([128, NB], f32, tag=f"z1_{c}", name=f"z1_{c}_{n}_{rep}")
                         for c in range(NCHUNK)]
                  for ko in range(KO):
                      for c in range(NCHUNK):
                          nc.tensor.matmul(
                              z1s[c][:],
                              lhsT=w1sb[ko][:, c, :],
                              rhs=sptsb[ko][:],
                              start=(ko == 0),
                              stop=(ko == KO - 1),
                          )
                  h1s = []
                  for c in range(NCHUNK):
                      h1 = h1p.tile([128, NB], mmdt, tag="h1")
                      if c % 2 == 0:
                          nc.scalar.activation(h1[:], z1s[c][:], relu,
                                               bias=float(b1f))
                      else:
                          nc.vector.tensor_scalar(
                              h1[:], z1s[c][:], float(b1f), 0.0,
                              mybir.AluOpType.add, mybir.AluOpType.max)
                      h1s.append(h1)

                  xs = []
                  for c in range(NCHUNK):
                      # z2 reuses z1_c's bank: the relu that freed it is
                      # exactly the producer of this conv's rhs.
                      z2 = ps1.tile([128, NB], f32, tag=f"z1_{c}",
                                    name=f"z2_{c}_{n}_{rep}")
                      nc.tensor.matmul(
                          z2[:], lhsT=c2sb[:, c, :], rhs=h1s[c][:],
                          start=True, stop=True,
                      )
                      x = xb.tile([128, NB], mmdt, tag="x")
                      if c % 2 == 1:
                          nc.scalar.activation(x[:], z2[:], relu,
                                               bias=float(b2f))
                      else:
                          nc.vector.tensor_scalar(
                              x[:], z2[:], float(b2f), 0.0,
                              mybir.AluOpType.add, mybir.AluOpType.max)
                      nc.scalar.dma_start(
                          xt_d[c, :, n * NB:(n + 1) * NB], x[:].bitcast(f32)
                      )
                      xs.append(x)

                  for mc in range(8):
                      o = pso.tile([128, NB], f32, tag="o")
                      for c in range(NCHUNK):
                          nc.tensor.matmul(
                              o[:],
                              lhsT=gpsb[c][:, mc, :],
                              rhs=xs[c][:],
                              start=(c == 0),
                              stop=(c == NCHUNK - 1),
                          )
                      osb = ob.tile([128, NB], f32, tag="osb")
                      nc.vector.tensor_copy(osb[:], o[:])
                      nc.sync.dma_start(
                          outt_d[mc, :, n * NB:(n + 1) * NB], osb[:]
                      )
    nc.finalize()
    return nc


def kernel(S, A, D, w1, b1, w2, b2, _trace=False, _tmpdir=None, _reps=1):
    from concourse.bass_utils import run_bass_kernel_spmd

    S = np.asarray(S)
    A = np.asarray(A, dtype=np.complex128)
    D = np.asarray(D, dtype=np.complex128)
    w1 = np.asarray(w1, dtype=np.float64)
    w2 = np.asarray(w2, dtype=np.float64)
    b1f = float(np.asarray(b1))
    b2f = float(np.asarray(b2))

    W1h, C2, GPh = _build_host_weights(A, D, w1, b1f, w2, b2f)
    SP = _pack_S(S)                                          # (B, 1024) f32

    # device-layout host arrays (shared across cores except spt)
    w1h_in = _tf32_round(np.ascontiguousarray(
        W1h.reshape(KO, 128, NCHUNK, 128).transpose(1, 0, 2, 3)
    ))                                                       # (128,KO,4,128)
    c2_in = _tf32_round(np.ascontiguousarray(C2.transpose(1, 0, 2)))
    gph_in = _tf32_round(np.ascontiguousarray(
        GPh.reshape(NCHUNK, 128, 8, 128).transpose(1, 0, 2, 3)
    ))                                                       # (128,4,8,128)

    in_maps = []
    for core in range(NCORES):
        sp_c = SP[core * BLOC:(core + 1) * BLOC]             # (BLOC, 1024)
        # spt[p, ko, b] = sp_c[b, ko*128 + p]
        spt = np.ascontiguousarray(
            sp_c.reshape(BLOC, KO, 128).transpose(2, 1, 0)
        )
        in_maps.append({"spt": _tf32_round(spt), "w1h": w1h_in, "c2": c2_in,
                        "gph": gph_in})

    nc = _build_program(b1f, b2f, reps=_reps)
    res = run_bass_kernel_spmd(
        nc, in_maps, core_ids=list(range(NCORES)),
        trace=_trace, tmpdir=_tmpdir,
    )

    # ---- host-side unshard / unpack ----
    iu = np.triu_indices(NCH, 1)
    x_full = np.empty((B, NPX), dtype=np.float64)
    out_full = np.empty((B, NCH, NCH), dtype=np.complex128)
    for core in range(NCORES):
        r = res.results[core]
        sl = slice(core * BLOC, (core + 1) * BLOC)
        xt = r["xt"]                                         # (4,128,BLOC)
        x_full[sl] = np.concatenate(
            [xt[c, :CH, :] for c in range(NCHUNK)], axis=0
        ).T
        outt = r["outt"].reshape(8 * 128, BLOC)              # (1024, BLOC)
        ob = np.empty((BLOC, NCH, NCH), dtype=np.complex128)
        ob[:, np.arange(NCH), np.arange(NCH)] = outt[:NCH].T
        re = outt[NCH:NCH + 496].T
        im = outt[NCH + 496:].T
        ob[:, iu[0], iu[1]] = re + 1j * im
        ob[:, iu[1], iu[0]] = re - 1j * im
        out_full[sl] = ob
    kernel._last_results = res
    return out_full, x_full

